# revision 57
# baseline (speedup 1.0000x reference)
"""Trainium2 Bass kernel for nn_AssociativeMemory (Hopfield recall).

Computes state <- tanh(W @ state) for 10 iterations, W: [8192, 8192] f32.

Strategy (8 NeuronCores, SPMD):
  - Row-shard W: core r owns rows [r*1024, (r+1)*1024).
  - fp16 hi/lo split of W and state (host-side for W): ~22 effective
    mantissa bits at 1 col/cycle PE rate (true fp32 matmul is 4x
    slower).
  - Scale design: Wl = fp16((W - Wh) * 2^6); A-pass stationary
    [sh, sl*2^12], B-pass stationary [sh*2^-6, sl*2^6].  A0/B0 share
    scale 1 and A1/B1 share scale 2^12, so all matmuls accumulate into
    ONE [2, 1024] PSUM tile; y = row0 + row1/2^12.  Every scale is a
    power of two and keeps all fp16 values normal (no subnormal flush).
  - Precision schedule: iterations 0-4 run hi+lo (22-bit W);
    iteration 5 runs lo on 32/64 chunks (bang-bang optimal: error
    injected at iteration t is amplified ~2x per remaining iteration,
    so precision is shed from the end); 6-9 run hi-only.  Measured
    1.037e-2 rel vs the 2e-2 gate — bit-deterministic run to run and
    matching the numpy simulation of the exact quantization schedule
    to <1%.
  - Pipelined halves: each iteration computes output half 0 then half
    1 (128 matmuls each).  Half-0's pre-activations AllGather while
    the PE computes half 1; the next iteration starts on the k-chunks
    half-0's gather delivered (k-halves map to chunk halves via
    k = r*1024 + h*512 + q*32 + c', chunk c = h*32 + c', partition
    p = r*16 + q), and reaches the half-1-dependent chunks after that
    gather lands.
  - Engine-queue discipline (FIFO queues => a blocked instruction
    blocks everything behind it):
      sync   : ALL W traffic (resident batched loads, streamed Wl) + x.
      gpsimd : collective triggers + half-1 cc_in/reload DMAs.
      scalar : half-0 cc_in/reload, tanh, half-1 PSUM copy.
      vector : half-0 PSUM copy, state combine + hi/lo splits.
    No W load can ever queue behind a collective-dependent op (the
    original version lost ~90us to exactly that at the first gather),
    and the two halves' blocking reloads live on different queues so
    neither gather chain can delay the other.
  - Warm-up AllGather at kernel start (input staged via scalar so its
    trigger is never queued behind anything) absorbs CC-core cold
    start and inter-core launch skew under the DMA-bound iteration 0.
  - Separate per-half PSUM accumulators; iteration-0 resident loads
    are full-width 4-chunk batches (2KB contiguous runs keep the DMA
    engines at ~90% of peak).
  - State stationaries are double-buffered across iterations; the
    s_b (lo-pass) stationary rows are skipped for hi-only consumers.
  - 64 Wh + RESIDENT_WL Wl chunk-units stay SBUF-resident across all
    10 iterations; the rest of Wl streams from HBM each full
    iteration in [128, 2chunk, 512] pieces.
"""

import numpy as np

import concourse.mybir as mybir
import concourse.tile as tile
from concourse import bacc
from concourse.bass_utils import run_bass_kernel_spmd

P = 8192
N_CORES = 8
ROWS = P // N_CORES          # 1024 output rows per core
NPART = 128                  # SBUF partitions / PE contraction size
CHUNKS = P // NPART          # 64 contraction chunks
HCHUNKS = CHUNKS // 2        # chunks per k-half
HALF = 512                   # output half width / PE moving free-dim
ITERATIONS = 10
SL_SCALE = 4096.0            # 2^12
WL_SCALE = 64.0              # 2^6
EPS = 1.0 / SL_SCALE

RESIDENT_WL = 26             # Wl chunks resident in SBUF (of 64)
# Residency split across the two k-halves; streamed pairs interleaved
# among resident chunks to spread HBM stream demand over each half.
RES_HALF = (14, 12)


def _wl_slot(c):
    """Resident-slot index for chunk c, or None if streamed."""
    h, cp = c // HCHUNKS, c % HCHUNKS
    if cp < RES_HALF[h]:
        return cp if h == 0 else RES_HALF[0] + cp
    return None


def _half_units(h):
    """Interleaved (kind, chunks) units for k-half h: 'r' = resident chunk,
    'p' = streamed adjacent pair."""
    base = h * HCHUNKS
    nres = RES_HALF[h]
    res = list(range(base, base + nres))
    streamed = list(range(base + nres, base + HCHUNKS))
    pairs = [(streamed[i], streamed[i + 1]) for i in range(0, len(streamed), 2)]
    units, ri, pi = [], 0, 0
    for _ in range(len(res) + len(pairs)):
        if pi >= len(pairs) or (ri < len(res) and ri * len(pairs) <= pi * len(res)):
            units.append(("r", (res[ri],)))
            ri += 1
        else:
            units.append(("p", pairs[pi]))
            pi += 1
    return units


def _iter0_units():
    """Plain-order units for iteration 0 (loads are batched separately)."""
    units = []
    c = 0
    while c < CHUNKS:
        if _wl_slot(c) is not None:
            units.append(("r", (c,)))
            c += 1
        else:
            units.append(("p", (c, c + 1)))
            c += 2
    return units


# Iterations that run the full hi+lo W product; later ones use Wh only.
# The last "full" iteration is PARTIAL: lo only on chunks < IT5_LO_CHUNKS
# (bang-bang optimal precision schedule; simulated rel err 1.04e-2 vs the
# 2e-2 gate -- the error is bit-deterministic run to run).  32 = exactly
# k-half 0, so the h0 pass keeps its full length and the k-half-1 state
# arrival budget is unchanged.
FULL_ITERS = 6
IT5_LO_CHUNKS = 32


def _has_lo(it, c):
    return it < FULL_ITERS - 1 or (it == FULL_ITERS - 1 and c < IT5_LO_CHUNKS)

_CACHED = {}


def _build_nc():
    # Bacc (not raw Bass): its generate_event_semaphores pass splits
    # multi-wait instructions (HW allows 1 wait/inst) via event semaphores.
    nc = bacc.Bacc(None, target_bir_lowering=False)
    f16 = mybir.dt.float16
    f32 = mybir.dt.float32

    xin = nc.dram_tensor("xin", [P], f32, kind="ExternalInput")
    wh = nc.dram_tensor("wh", [NPART, CHUNKS, ROWS], f16, kind="ExternalInput")
    wl = nc.dram_tensor("wl", [NPART, CHUNKS, ROWS], f16, kind="ExternalInput")
    # each core writes only its own row-slice; the host concatenates
    out = nc.dram_tensor("out", [ROWS], f32, kind="ExternalOutput")
    # [1, eps] column for the final-iteration PSUM row combine on the PE
    cvec = nc.inline_tensor(np.array([[1.0], [EPS]], dtype=np.float32), name="cvec")

    with tile.TileContext(nc) as tc:
        with (
            tc.tile_pool(name="wres", bufs=1) as wres,
            tc.tile_pool(name="stream", bufs=9) as stream,
            tc.tile_pool(name="state", bufs=1) as state,
            tc.tile_pool(name="tmp", bufs=2) as tmp,
            tc.tile_pool(name="psum", bufs=2, space="PSUM") as psum,
            tc.tile_pool(name="dram", bufs=1, space="DRAM") as dram,
        ):
            # resident weights; batched loads issued inside iteration 0
            wh_sb = wres.tile([NPART, CHUNKS, ROWS], f16)
            wl_sb = wres.tile([NPART, RESIDENT_WL, ROWS], f16)

            # state stationaries, double-buffered across iterations:
            # s_a[b] = [sh, sl*2^12], s_b[b] = [sh*2^-6, sl*2^6]
            s_a = [state.tile([NPART, 2, CHUNKS], f16, name=f"s_a{b}") for b in (0, 1)]
            s_b = [state.tile([NPART, 2, CHUNKS], f16, name=f"s_b{b}") for b in (0, 1)]

            def split_state(src_f32, buf, csl, need_b=True):
                """hi/lo split of [128, n] state into chunk-slice csl of buf.
                s_b feeds only the lo pass; skip it for hi-only consumers."""
                d_full = tmp.tile([NPART, CHUNKS], f32, tag="d", name="d_full")
                d = d_full[:, csl]
                sa, sb = s_a[buf], s_b[buf]
                nc.vector.tensor_copy(sa[:, 0, csl], src_f32[:])
                nc.vector.tensor_tensor(
                    d, src_f32[:], sa[:, 0, csl], mybir.AluOpType.subtract
                )
                nc.vector.tensor_scalar_mul(sa[:, 1, csl], d, SL_SCALE)
                if need_b:
                    nc.vector.tensor_scalar_mul(
                        sb[:, 0, csl], sa[:, 0, csl], 1.0 / WL_SCALE
                    )
                    nc.vector.tensor_scalar_mul(sb[:, 1, csl], d, WL_SCALE)

            # initial split of x into buffer 0 (no tanh on iteration-1 input)
            x_sb = state.tile([NPART, CHUNKS], f32)
            nc.sync.dma_start(x_sb[:], xin.rearrange("(p c) -> p c", p=NPART))
            split_state(x_sb, 0, slice(0, CHUNKS))

            cvec_sb = state.tile([2, 1], f32)
            nc.scalar.dma_start(cvec_sb[:], cvec[:])

            # Warm-up AllGather: absorbs CC-core cold-start (~11us) and
            # inter-core launch skew while iteration 0 is DMA-bound, so
            # the first real gather runs at steady-state latency.
            warm_in = dram.tile([1, 1], f32, name="warm_in")
            warm_out = dram.tile(
                [N_CORES, 1, 1], f32, addr_space="Shared", name="warm_out"
            )
            nc.scalar.dma_start(warm_in[:], cvec_sb[0:1, 0:1])
            nc.gpsimd.collective_compute(
                "AllGather",
                mybir.AluOpType.bypass,
                replica_groups=[list(range(N_CORES))],
                ins=[warm_in[:]],
                outs=[warm_out[:]],
            )

            def iter0_loads():
                """Batched FULL-WIDTH resident loads for iteration 0 on the
                sync queue, in chunk order (2KB contiguous runs per
                partition keep the DMA engines near peak).  Streamed-Wl
                pairs (also full-width: iter 0 consumes both halves per
                chunk) are interleaved at their consumption position;
                returns {pair_c0: stream_tile} for the MM loop."""
                f16_ = mybir.dt.float16
                pair_tiles = {}
                for c0 in range(0, CHUNKS, 4):
                    if c0 == 0:  # small first batch: earliest possible MM
                        nc.sync.dma_start(wh_sb[:, 0:2, :], wh[:, 0:2, :])
                        nc.sync.dma_start(wh_sb[:, 2:4, :], wh[:, 2:4, :])
                    else:
                        nc.sync.dma_start(
                            wh_sb[:, c0 : c0 + 4, :], wh[:, c0 : c0 + 4, :]
                        )
                    run = [c for c in range(c0, c0 + 4) if _wl_slot(c) is not None]
                    if run:
                        s0, cA = _wl_slot(run[0]), run[0]
                        n = len(run)
                        nc.sync.dma_start(
                            wl_sb[:, s0 : s0 + n, :], wl[:, cA : cA + n, :]
                        )
                    for c in range(c0, c0 + 4):
                        if _wl_slot(c) is None and c % 2 == 0:
                            wl_t = stream.tile([NPART, 2, HALF], f16_, tag="wl_t")
                            nc.sync.dma_start(wl_t[:], wl[:, c : c + 2, 0:HALF])
                            pair_tiles[c] = wl_t
                return pair_tiles

            def gather_tail(it, h, acc):
                """AllGather output-half h of iteration `it`, then tanh and
                split into state buffer (it+1)%2 chunk-half h (hidden under
                subsequent matmuls).  The final iteration skips the gather:
                combine rows on the PE, tanh, and write this core's slice."""
                osl = slice(h * HALF, (h + 1) * HALF)
                u_sb = tmp.tile([2, HALF], f32, tag="u_sb")
                # h0's copy on vector, h1's on scalar: decoupled queues so
                # neither copy can wait behind the other half's chain.
                if h == 0:
                    nc.vector.tensor_copy(u_sb[:], acc[:])
                else:
                    nc.scalar.activation(
                        u_sb[:], acc[:], mybir.ActivationFunctionType.Copy
                    )
                if it == ITERATIONS - 1:
                    yf = psum.tile([1, HALF], f32, tag="yf")
                    nc.tensor.matmul(yf[:], cvec_sb[:], u_sb[:], start=True, stop=True)
                    yt = tmp.tile([1, HALF], f32, tag="yt", bufs=1)
                    nc.scalar.activation(
                        yt[:], yf[:], mybir.ActivationFunctionType.Tanh
                    )
                    nc.scalar.dma_start(out.rearrange("(a b) -> a b", a=1)[:, osl], yt[:])
                    return
                # cc_in holds this core's contribution permuted to
                # [q, j, c'] so the gathered [r, q, j, c'] buffer merges
                # into a uniform-stride (r q) partition dim on reload.
                q16 = NPART // N_CORES
                cc_in = dram.tile([q16, 2, HCHUNKS], f32, name=f"cc_in_{it}_{h}")
                cc_out = dram.tile(
                    [N_CORES, q16, 2, HCHUNKS], f32, addr_space="Shared",
                    name=f"cc_out_{it}_{h}",
                )
                # Collective triggers stay on gpsimd (NRT straight-line
                # ordering).  Both halves' cc_in DMAs go on scalar: h1's
                # lands right behind its copy (same queue, no cross-engine
                # hop).  The blocking reloads alternate between scalar
                # (h0) and gpsimd+scalar (h1, split) so one half's reload
                # (which waits on its collective) never delays the other
                # half's chain, and tanh(h0) (scalar) is never stuck
                # behind a blocking reload.
                deng = nc.scalar if h == 0 else nc.gpsimd
                nc.scalar.dma_start(cc_in.rearrange("q j c -> j q c"), u_sb[:])
                nc.gpsimd.collective_compute(
                    "AllGather",
                    mybir.AluOpType.bypass,
                    replica_groups=[list(range(N_CORES))],
                    ins=[cc_in[:]],
                    outs=[cc_out[:]],
                )
                # reload at [128, 2, 32] (k = r*1024 + h*512 + q*32 + c'
                # ->  partition r*16+q).  h1's reload splits into two
                # parallel rank-group DMAs (gpsimd+scalar); h0's stays a
                # single DMA on scalar so it cannot delay h1's cc_in on
                # the gpsimd queue.
                u2 = tmp.tile([NPART, 2, HCHUNKS], f32, tag="u2")
                if h == 0:
                    deng.dma_start(u2[:], cc_out.rearrange("r q j c -> (r q) j c"))
                else:
                    deng.dma_start(
                        u2[0:64, :, :],
                        cc_out[0:4].rearrange("r q j c -> (r q) j c"),
                    )
                    nc.scalar.dma_start(
                        u2[64:128, :, :],
                        cc_out[4:8].rearrange("r q j c -> (r q) j c"),
                    )
                s_pre = tmp.tile([NPART, HCHUNKS], f32, tag="s_pre")
                nc.vector.scalar_tensor_tensor(
                    s_pre[:],
                    u2[:, 1, :],
                    EPS,
                    u2[:, 0, :],
                    mybir.AluOpType.mult,
                    mybir.AluOpType.add,
                )
                s_f = tmp.tile([NPART, HCHUNKS], f32, tag="s_f")
                nc.scalar.activation(
                    s_f[:], s_pre[:], mybir.ActivationFunctionType.Tanh
                )
                csl = slice(h * HCHUNKS, (h + 1) * HCHUNKS)
                split_state(s_f, (it + 1) % 2, csl, need_b=(it + 1 < FULL_ITERS))

            for it in range(ITERATIONS):
                buf = it % 2
                it0_pairs = iter0_loads() if it == 0 else None
                for h in range(2):
                    acc = psum.tile([2, HALF], f32, tag=f"acc{h}")
                    osl = slice(h * HALF, (h + 1) * HALF)
                    full = it < FULL_ITERS
                    if it == 0:
                        units = _iter0_units()
                    else:
                        # k-half 0 units first (that gather has landed; the
                        # half-1 gather lands before the PE reaches them).
                        units = _half_units(0) + _half_units(1)
                    seq = [c for _, cs in units for c in cs]
                    cfirst, clast = seq[0], seq[-1]
                    for kind, chunks in units:
                        if kind == "p" and _has_lo(it, chunks[0]):
                            c0 = chunks[0]
                            if it == 0 and h == 0:
                                wl_t = it0_pairs[c0]  # preloaded, interleaved
                            else:
                                wl_t = stream.tile([NPART, 2, HALF], f16, tag="wl_t")
                                nc.sync.dma_start(wl_t[:], wl[:, c0 : c0 + 2, osl])
                        for j, c in enumerate(chunks):
                            lo_c = _has_lo(it, c)
                            if not lo_c:
                                wl_rhs = None
                            elif kind == "r":
                                wl_rhs = wl_sb[:, _wl_slot(c), osl]
                            else:
                                wl_rhs = wl_t[:, j, :]
                            first = c == cfirst
                            last = c == clast
                            nc.tensor.matmul(
                                acc[:],
                                s_a[buf][:, :, c],
                                wh_sb[:, c, osl],
                                start=first,
                                stop=(last and not lo_c),
                            )
                            if lo_c:
                                nc.tensor.matmul(
                                    acc[:],
                                    s_b[buf][:, :, c],
                                    wl_rhs,
                                    start=False,
                                    stop=last,
                                )
                    gather_tail(it, h, acc)
    nc.compile()
    return nc


def _prepare_in_maps(x, weights):
    x = np.ascontiguousarray(x, dtype=np.float32)
    w32 = np.asarray(weights, dtype=np.float32)
    # k-map: k = r*1024 + h*512 + q*32 + c'  <->  p = r*16+q, c = h*32+c'
    in_maps = []
    for r in range(N_CORES):
        wt = np.ascontiguousarray(w32[r * ROWS : (r + 1) * ROWS, :].T)  # [8192, 1024]
        whi = wt.astype(np.float16)
        wlo = ((wt - whi.astype(np.float32)) * WL_SCALE).astype(np.float16)

        def remap(a):
            # [8192 k, 1024 i] -> [128 p, 64 c, 1024 i]
            a = a.reshape(N_CORES, 2, NPART // N_CORES, HCHUNKS, ROWS)
            a = a.transpose(0, 2, 1, 3, 4)  # r, q, h, c', i
            return np.ascontiguousarray(a.reshape(NPART, CHUNKS, ROWS))

        in_maps.append({"xin": _permute_x(x), "wh": remap(whi), "wl": remap(wlo)})
    return in_maps


def _permute_x(x):
    # xin DMA loads [p, c] as x[p*64 + c]; give it x in the k-map order:
    # position p*64+c must hold x[k(p, c)]
    k = _kmap()
    return np.ascontiguousarray(x[k].reshape(-1))


def _kmap():
    p = np.arange(NPART)[:, None]
    c = np.arange(CHUNKS)[None, :]
    r, q = p // 16, p % 16
    h, cp = c // HCHUNKS, c % HCHUNKS
    return (r * ROWS + h * HALF + q * HCHUNKS + cp).reshape(NPART, CHUNKS)


def _run(inputs, **kwargs):
    if "nc" not in _CACHED:
        _CACHED["nc"] = _build_nc()
    nc = _CACHED["nc"]
    in_maps = _prepare_in_maps(inputs["x"], inputs["weights"])
    last_exc = None
    for _ in range(3):  # retry transient device/load hiccups
        try:
            res = run_bass_kernel_spmd(
                nc, in_maps, core_ids=list(range(N_CORES)), **kwargs
            )
            break
        except Exception as e:  # noqa: BLE001
            last_exc = e
    else:
        raise last_exc
    out = np.concatenate([np.asarray(res.results[r]["out"]) for r in range(N_CORES)])
    return np.ascontiguousarray(out, dtype=np.float32), res


def kernel(**inputs) -> np.ndarray:
    out, _ = _run(inputs)
    return out
